# revision 11
# baseline (speedup 1.0000x reference)
"""Trainium2 Bass kernel for nn_DeformableAttention (B=4, C=384, H=W=56, NH=12, HC=32, STRIDE=2).

Self-contained. Sharding: 8 cores = 4 batches x 2 pixel-row-halves. Each core
computes the full value/key/off branches for its batch (duplicated across the
pair) and the query branch + final GEMM for its half of the 3136 output pixels.

v2 restructure vs the original baseline:
- x is cast to f16 on the host; weight diagonals for PE convs are host-built.
- value/key/off dwconvs run on the PE as diagonal-weight matmuls (9 taps
  accumulated in PSUM, <=512-col chunks); q conv stays on DVE.
- value conv output is kept row-pitch-58 ("pitched") so PE conv rhs slices are
  flat/contiguous; gather indices use the same pitch.
- The pixel-major value table is a PAIR table (row p = val[p] | val[p+1]) so
  each sampled point needs 2 gathers instead of 4; bilinear is factored into
  2 ACT weighted copies + 2 DVE fused multiply-adds.
- The vtab transposes are split between PE (identity matmuls) and the two
  HWDGE xbar-transpose queues; writes are batched 4 chunks per DMA.
- LN rstd via ACT Ln/Exp (no slow [1,784] DVE reciprocal); LN stats and
  broadcasts via f16 PE matmuls.
- M = V K^T via 21 PSUM-accumulated [112]x[128,128] matmuls.
"""
import contextlib

import numpy as np

import concourse.bass as bass
import concourse.tile as tile
from concourse import bacc, mybir
from concourse.bass_utils import run_bass_kernel_spmd
from concourse.masks import make_identity

F32, F16, I32 = mybir.dt.float32, mybir.dt.float16, mybir.dt.int32
MULT, ADD, SUB = mybir.AluOpType.mult, mybir.AluOpType.add, mybir.AluOpType.subtract
AF = mybir.ActivationFunctionType

B, C, H, W = 4, 384, 56, 56
NH, HC = 12, 32
SCALE = HC ** -0.5
HP = H + 2                      # 58 padded
PIT = HP                        # value row pitch
VPIX = H * PIT                  # 3248 pitched pixels
VPAD = 3328                     # padded to 26*128
KH = KW = 28
N = KH * KW                     # 784
NT = 112
NTILES = N // NT
HALF_ROWS = H // 2
HALF_PIX = HALF_ROWS * W        # 1568
CT = C // 128
EPS = 1e-5
NCHUNK_V = 7                    # value conv: 7 chunks of 8 rows (464 cols)
PE_VAL_CT = (0, 1, 2)           # value-conv channel tiles on PE
XBAR_CHUNKS = 8                 # vtab chunks transposed via DMA xbar (of 26)

_CACHE = {}


def _emit(nc, tc, ctx, io):
    pool = ctx.enter_context(tc.tile_pool(name="main", bufs=1))
    q0, q1 = nc.sync, nc.scalar

    # ---------------- loads ----------------
    xs16 = []
    for ct in range(CT):
        t = pool.tile([128, HP * HP], F16, tag=f"xs16_{ct}")
        q0.dma_start(t[:], io["x16"][ct * 128:(ct + 1) * 128, :])
        xs16.append(t)
    xp32pool = tc.tile_pool(name="xp32p", bufs=1)
    xp32p = xp32pool.__enter__()
    xp32 = []
    for ct in range(CT):
        t = xp32p.tile([128, HP * HP], F32, tag=f"xp32_{ct}", name=f"xp32_{ct}")
        q0.dma_start(t[:], io["x32"][ct * 128:(ct + 1) * 128, :])
        xp32.append(t)
    diag3 = []
    for ct in range(CT):
        t = pool.tile([128, 18 * 128], F16, tag=f"diag3_{ct}")
        q1.dma_start(t[:], io["diag3"][ct * 128:(ct + 1) * 128, :])
        diag3.append(t)
    misc = []
    for ct in range(CT):
        t = pool.tile([128, 34], F32, tag=f"misc_{ct}")
        q0.dma_start(t[:], io["misc"][ct * 128:(ct + 1) * 128, :])
        misc.append(t)
    # misc columns: 0-8 wq, 9 bq, 10 bv, 11 bo, 12 bkS, 13 lng, 14 lnb
    wot16 = []
    for ct in range(CT):
        t = pool.tile([128, C], F16, tag=f"wot16_{ct}")
        q1.dma_start(t[:], io["wot16"][ct * 128:(ct + 1) * 128, :])
        wot16.append(t)
    w2t32 = []
    for ct in range(CT):
        t = pool.tile([128, 2], F32, tag=f"w2t32_{ct}")
        q0.dma_start(t[:], io["w2t32"][ct * 128:(ct + 1) * 128, :])
        w2t32.append(t)
    refyx = pool.tile([2, N], F32, tag="refyx")
    q0.dma_start(refyx[:], io["refyx"][:, :])

    ones16 = pool.tile([128, 1], F16, tag="ones16")
    nc.vector.memset(ones16[:], 1.0)
    onerow32 = pool.tile([1, 128], F32, tag="onerow32")
    nc.vector.memset(onerow32[:], 1.0)
    ident = pool.tile([128, 128], F16, tag="ident")
    make_identity(nc, ident[:])

    def dval(ct, t):
        return diag3[ct][:, t * 128:(t + 1) * 128]

    def dkey(ct, t):
        return diag3[ct][:, (9 + t) * 128:(10 + t) * 128]

    # ---------------- off conv (DVE, f32) + LN stats (f16 PE) ----------------
    off32, off16, sq16 = [], [], []
    for ct in range(CT):
        x3 = xp32[ct][:].rearrange("p (h w) -> p h w", h=HP)
        o32 = pool.tile([128, N], F32, tag=f"off32_{ct}")
        o3 = o32[:].rearrange("p (h w) -> p h w", h=KH)
        for t in range(9):
            dy, dx = t // 3, t % 3
            srcv = x3[:, dy:dy + 55:2, dx:dx + 55:2]
            if t == 0:
                nc.vector.tensor_scalar(out=o3, in0=srcv,
                                        scalar1=misc[ct][:, 11:12],
                                        scalar2=None, op0=MULT)
            else:
                nc.vector.scalar_tensor_tensor(out=o3, in0=srcv,
                                               scalar=misc[ct][:, 11 + t:12 + t],
                                               in1=o3, op0=MULT, op1=ADD)
        off32.append(o32)
        o16 = pool.tile([128, N], F16, tag=f"off16_{ct}")
        nc.scalar.activation(o16[:], o32[:], AF.Copy)
        off16.append(o16)
        s16 = pool.tile([128, N], F16, tag=f"sq16_{ct}")
        nc.scalar.activation(s16[:], o32[:], AF.Square)
        sq16.append(s16)
    # ---------------- value conv (PE, pitched) ----------------
    val = []
    for ct in range(CT):
        t = pool.tile([128, VPAD], F16, tag=f"val_{ct}", name=f"val_{ct}")
        nc.vector.memset(t[:, VPIX - 2:], 0.0)
        val.append(t)
    xflat = [xs16[ct][:] for ct in range(CT)]
    valp = ctx.enter_context(tc.tile_pool(name="valps", bufs=1, space="PSUM"))
    val_pss = [valp.tile([128, 8 * PIT], F32, tag=f"val_ps{i}", name=f"val_ps{i}")
               for i in range(2)]

    def emit_valconv(ct):
        for c in range(NCHUNK_V):
            ps = val_pss[c % 2]
            r0 = c * 8
            cw = 8 * PIT if c < NCHUNK_V - 1 else 8 * PIT - 2
            for t in range(9):
                dy, dx = t // 3, t % 3
                base = (r0 + dy) * PIT + dx
                nc.tensor.matmul(ps[:, :cw], dval(ct, t),
                                 xflat[ct][:, base:base + cw],
                                 start=(t == 0), stop=(t == 8))
            nc.scalar.activation(val[ct][:, r0 * PIT:r0 * PIT + cw],
                                 ps[:, :cw], AF.Identity,
                                 bias=misc[ct][:, 10:11])

    xp32pool.__exit__(None, None, None)

    emit_valconv(0)
    emit_valconv(1)

    with tc.tile_pool(name="statps", bufs=1, space="PSUM") as statp:
        mu_ps = statp.tile([1, N], F32, tag="mu_ps")
        ssq_ps = statp.tile([1, N], F32, tag="ssq_ps")
        for sl in (slice(0, 512), slice(512, N)):
            for ct in range(CT):
                nc.tensor.matmul(mu_ps[:, sl], ones16[:], off16[ct][:, sl],
                                 start=(ct == 0), stop=(ct == CT - 1))
            for ct in range(CT):
                nc.tensor.matmul(ssq_ps[:, sl], ones16[:], sq16[ct][:, sl],
                                 start=(ct == 0), stop=(ct == CT - 1))
        mu32 = pool.tile([1, N], F32, tag="mu32")
        nc.scalar.activation(mu32[:], mu_ps[:], AF.Copy, scale=1.0 / C)
        es32 = pool.tile([1, N], F32, tag="es32")
        nc.scalar.activation(es32[:], ssq_ps[:], AF.Copy, scale=1.0 / C)
    musq = pool.tile([1, N], F32, tag="musq")
    nc.vector.tensor_tensor(out=musq[:], in0=mu32[:], in1=mu32[:], op=MULT)
    var = pool.tile([1, N], F32, tag="var")
    nc.vector.tensor_tensor(out=var[:], in0=es32[:], in1=musq[:], op=SUB)
    nc.vector.tensor_scalar_add(var[:], var[:], EPS)
    lnv = pool.tile([1, N], F32, tag="lnv")
    nc.scalar.activation(lnv[:], var[:], AF.Ln)
    rstd32 = pool.tile([1, N], F32, tag="rstd32")
    nc.scalar.activation(rstd32[:], lnv[:], AF.Exp, scale=-0.5)
    # ---------------- query conv (DVE) ----------------
    xq = []
    for ct in range(CT):
        t = pool.tile([128, 30 * HP], F16, tag=f"xq_{ct}")
        q1.dma_start(t[:], io["xq16"][ct * 128:(ct + 1) * 128, :])
        xq.append(t)
    q16 = [pool.tile([128, HALF_PIX], F16, tag=f"q_{ct}", name=f"q16_{ct}")
           for ct in range(CT)]

    def emit_qconv(ct):
        x3 = xq[ct][:].rearrange("p (h w) -> p h w", h=30)
        o3 = q16[ct][:].rearrange("p (h w) -> p h w", h=HALF_ROWS)
        for tt in range(9):
            dy, dx = tt // 3, tt % 3
            srcv = x3[:, dy:dy + HALF_ROWS, dx:dx + W]
            if tt == 0:
                nc.vector.tensor_scalar(out=o3, in0=srcv,
                                        scalar1=misc[ct][:, 0:1],
                                        scalar2=misc[ct][:, 9:10],
                                        op0=MULT, op1=ADD)
            else:
                nc.vector.scalar_tensor_tensor(out=o3, in0=srcv,
                                               scalar=misc[ct][:, tt:tt + 1],
                                               in1=o3, op0=MULT, op1=ADD)

    emit_qconv(0)

    # broadcast mu/rstd across partitions (K=1 PE matmul), f32
    mu_b = pool.tile([128, N], F32, tag="mu_b")
    rstd_b = pool.tile([128, N], F32, tag="rstd_b")
    with tc.tile_pool(name="bcps", bufs=1, space="PSUM") as bcp:
        bc_ps = bcp.tile([128, N], F32, tag="bc_ps")
        for sl in (slice(0, 512), slice(512, N)):
            nc.tensor.matmul(bc_ps[:, sl], onerow32[:], mu32[:, sl],
                             start=True, stop=True)
        nc.scalar.activation(mu_b[:], bc_ps[:], AF.Copy)
        for sl in (slice(0, 512), slice(512, N)):
            nc.tensor.matmul(bc_ps[:, sl], onerow32[:], rstd32[:, sl],
                             start=True, stop=True)
        nc.scalar.activation(rstd_b[:], bc_ps[:], AF.Copy)

    # ---------------- LN norm + gelu + off2 + tanh + ixy ----------------
    gel = []
    for ct in range(CT):
        t1 = pool.tile([128, N], F32, tag=f"t1_{ct}")
        nc.vector.tensor_tensor(out=t1[:], in0=off32[ct][:], in1=mu_b[:], op=SUB)
        nc.vector.scalar_tensor_tensor(out=t1[:], in0=t1[:],
                                       scalar=misc[ct][:, 21:22],
                                       in1=rstd_b[:], op0=MULT, op1=MULT)
        g = off32[ct]  # reuse as f32 gelu output
        nc.scalar.activation(g[:], t1[:], AF.Gelu, bias=misc[ct][:, 22:23])
        gel.append(g)

    pos = pool.tile([2, N], F32, tag="pos")
    with tc.tile_pool(name="offps2", bufs=1, space="PSUM") as offp2:
        oyx_ps = offp2.tile([2, N], F32, tag="oyx")
        for sl in (slice(0, 512), slice(512, N)):
            for ct in range(CT):
                nc.tensor.matmul(oyx_ps[:, sl], w2t32[ct][:], gel[ct][:, sl],
                                 start=(ct == 0), stop=(ct == CT - 1))
        nc.vector.tensor_tensor(out=pos[:], in0=oyx_ps[:], in1=refyx[:], op=ADD)
    nc.scalar.activation(pos[:], pos[:], AF.Tanh)
    ixy = pool.tile([2, N], F32, tag="ixy")
    nc.vector.tensor_scalar(out=ixy[:], in0=pos[:], scalar1=(H - 1) / 2.0,
                            scalar2=(H - 1) / 2.0, op0=MULT, op1=ADD)
    ixy_write = q0.dma_start(io["ixy_dram"][:, :], ixy[:])

    # ---------------- index math (early; gates the gathers) ----------------
    idx_omf, idx_frac, idx_i = [], [], []
    idxp = ctx.enter_context(tc.tile_pool(name="idxp", bufs=NTILES))
    for k in range(NTILES):
        iy_x = idxp.tile([NT, 2], F32, tag="iyx", name=f"iyx_{k}")
        src = bass.AP(io["ixy_dram"].tensor, k * NT, [[1, NT], [N, 2]])
        rd = q0.dma_start(iy_x[:], src)
        tile.add_dep_helper(rd.ins, ixy_write.ins, reason="ixy dram RAW")
        xy0i = idxp.tile([NT, 2], I32, tag="xy0i", name=f"xy0i_{k}")
        nc.vector.tensor_copy(xy0i[:], iy_x[:])
        xy0f = idxp.tile([NT, 2], F32, tag="xy0f", name=f"xy0f_{k}")
        nc.vector.tensor_copy(xy0f[:], xy0i[:])
        gtm = idxp.tile([NT, 2], F32, tag="gtm", name=f"gtm_{k}")
        nc.vector.tensor_tensor(out=gtm[:], in0=xy0f[:], in1=iy_x[:],
                                op=mybir.AluOpType.is_gt)
        nc.vector.tensor_tensor(out=xy0f[:], in0=xy0f[:], in1=gtm[:], op=SUB)
        nc.vector.tensor_scalar(out=xy0f[:], in0=xy0f[:], scalar1=float(H - 2),
                                scalar2=None, op0=mybir.AluOpType.min)
        frac = idxp.tile([NT, 2], F32, tag="frac", name=f"frac_{k}")
        nc.vector.tensor_tensor(out=frac[:], in0=iy_x[:], in1=xy0f[:], op=SUB)
        omf = idxp.tile([NT, 2], F32, tag="omf", name=f"omf_{k}")
        nc.vector.tensor_scalar(out=omf[:], in0=frac[:], scalar1=-1.0,
                                scalar2=1.0, op0=MULT, op1=ADD)
        idxf = idxp.tile([NT, 1], F32, tag="idxf", name=f"idxf_{k}")
        nc.vector.scalar_tensor_tensor(out=idxf[:], in0=xy0f[:, 0:1],
                                       scalar=float(PIT), in1=xy0f[:, 1:2],
                                       op0=MULT, op1=ADD)
        idxi = idxp.tile([NT, 2], I32, tag="idxi", name=f"idxi_{k}")
        nc.vector.tensor_copy(idxi[:, 0:1], idxf[:])
        nc.vector.tensor_scalar_add(idxf[:], idxf[:], float(PIT))
        nc.vector.tensor_copy(idxi[:, 1:2], idxf[:])
        idx_omf.append(omf); idx_frac.append(frac); idx_i.append(idxi)


    emit_valconv(2)

    # ---------------- vtab2 pair table ----------------
    # 26 chunks of 128 pitched pixels -> wide4 groups of 4 chunks
    wr_insts = []
    with tc.tile_pool(name="wide", bufs=3) as widep, \
         tc.tile_pool(name="tpps", bufs=2, space="PSUM") as tpp:
        for g in range(7):
            nch = 4 if g < 6 else 2
            w4 = widep.tile([128, 4, C], F16, tag="wide4", name=f"wide4_{g}")
            for j in range(nch):
                chunk = g * 4 + j
                use_xbar = chunk < XBAR_CHUNKS
                for ct in range(CT):
                    src = val[ct][:, chunk * 128:(chunk + 1) * 128]
                    if use_xbar:
                        eng = q0 if (chunk * 3 + ct) % 2 == 0 else q1
                        eng.dma_start_transpose(w4[:, j, ct * 128:(ct + 1) * 128], src)
                    else:
                        ps = tpp.tile([128, 128], F16, tag="tp_ps", space="PSUM")
                        nc.tensor.transpose(ps[:], src, ident[:])
                        nc.vector.tensor_scalar(
                            out=w4[:, j, ct * 128:(ct + 1) * 128], in0=ps[:],
                            scalar1=1.0, scalar2=None, op0=MULT)
            # write A: rows g*512 + 128j + p, first half of vtab2 row
            dstA = bass.AP(io["vtab2"].tensor, (g * 512) * 2 * C,
                           [[2 * C, 128], [128 * 2 * C, nch], [1, C]])
            wr_insts.append(q0.dma_start(dstA, w4[:, 0:nch, :]))
            # write B: rows g*512 + 128j + p - 1, second half
            if g == 0:
                dstB = bass.AP(io["vtab2"].tensor, 0 * 2 * C + C,
                               [[2 * C, 127], [128 * 2 * C, 1], [1, C]])
                wr_insts.append(q1.dma_start(dstB, w4[1:128, 0, :]))
                dstB2 = bass.AP(io["vtab2"].tensor, 127 * 2 * C + C,
                                [[2 * C, 128], [128 * 2 * C, nch - 1], [1, C]])
                wr_insts.append(q1.dma_start(dstB2, w4[:, 1:nch, :]))
            else:
                dstB = bass.AP(io["vtab2"].tensor, (g * 512 - 1) * 2 * C + C,
                               [[2 * C, 128], [128 * 2 * C, nch], [1, C]])
                wr_insts.append(q1.dma_start(dstB, w4[:, 0:nch, :]))

    # ---------------- key conv (PE, stride-2) + kT ----------------
    key16 = []
    with tc.tile_pool(name="keyps", bufs=2, space="PSUM") as keyp:
        for ct in range(CT):
            x3 = xs16[ct][:].rearrange("p (h w) -> p h w", h=HP)
            t = pool.tile([128, N], F16, tag=f"key_{ct}")
            for c in range(2):
                ps = keyp.tile([128, 14, 28], F32, tag="key_ps", space="PSUM")
                for tt in range(9):
                    dy, dx = tt // 3, tt % 3
                    nc.tensor.matmul(ps[:], dkey(ct, tt),
                                     x3[:, dy + 28 * c:dy + 28 * c + 27:2,
                                        dx:dx + 55:2],
                                     start=(tt == 0), stop=(tt == 8))
                nc.scalar.activation(t[:, c * 392:(c + 1) * 392],
                                     ps[:].rearrange("p a b -> p (a b)"),
                                     AF.Identity, scale=SCALE,
                                     bias=misc[ct][:, 20:21])
            key16.append(t)
    kT = []
    with tc.tile_pool(name="ktp", bufs=2, space="PSUM") as ktp:
        for k in range(NTILES):
            t = pool.tile([NT, C], F16, tag=f"kT_{k}")
            for ct in range(CT):
                ps = ktp.tile([NT, 128], F16, tag="kt_ps", space="PSUM")
                nc.tensor.transpose(ps[:], key16[ct][:, k * NT:(k + 1) * NT], ident[:])
                nc.vector.tensor_scalar(out=t[:, ct * 128:(ct + 1) * 128],
                                        in0=ps[:], scalar1=1.0, scalar2=None,
                                        op0=MULT)
            kT.append(t)

    emit_qconv(1)

    # ---------------- pair gathers + factored bilinear ----------------
    vs = []
    with tc.tile_pool(name="gat", bufs=4) as gat:
        for k in range(NTILES):
            omf, frac, idxi = idx_omf[k], idx_frac[k], idx_i[k]
            g0 = gat.tile([NT, 2 * C], F16, tag="g0", name=f"g0_{k}")
            g1 = gat.tile([NT, 2 * C], F16, tag="g1", name=f"g1_{k}")
            for j, gt in enumerate((g0, g1)):
                gi = nc.gpsimd.indirect_dma_start(
                    out=gt[:], out_offset=None, in_=io["vtab2"][:, :],
                    in_offset=bass.IndirectOffsetOnAxis(ap=idxi[:, j:j + 1], axis=0),
                    element_offset=0,
                    bounds_check=VPAD - 1, oob_is_err=False)
                for wi in wr_insts:
                    tile.add_dep_helper(gi.ins, wi.ins, reason="vtab2 RAW")
            tmix = gat.tile([NT, 2 * C], F16, tag="tmix", name=f"tmix_{k}")
            nc.scalar.activation(tmix[:], g0[:], AF.Copy, scale=omf[:, 0:1])
            nc.vector.scalar_tensor_tensor(out=tmix[:], in0=g1[:],
                                           scalar=frac[:, 0:1], in1=tmix[:],
                                           op0=MULT, op1=ADD)
            v = pool.tile([NT, C], F16, tag=f"vs_{k}")
            nc.scalar.activation(v[:], tmix[:, 0:C], AF.Copy, scale=omf[:, 1:2])
            nc.vector.scalar_tensor_tensor(out=v[:], in0=tmix[:, C:2 * C],
                                           scalar=frac[:, 1:2], in1=v[:],
                                           op0=MULT, op1=ADD)
            vs.append(v)

    emit_qconv(2)

    # ---------------- M (PSUM-accumulated), m16, at, y ----------------
    m16 = [pool.tile([128, HC], F16, tag=f"m16_{i}", name=f"m16t_{i}")
           for i in range(CT)]
    with tc.tile_pool(name="mps", bufs=1, space="PSUM") as mps:
        m_ps = [mps.tile([128, 128], F32, tag=f"m_ps{i}", name=f"m_ps{i}")
                for i in range(CT)]
        for k in range(NTILES):
            for ct in range(CT):
                nc.tensor.matmul(m_ps[ct][:], vs[k][:, ct * 128:(ct + 1) * 128],
                                 kT[k][:, ct * 128:(ct + 1) * 128],
                                 start=(k == 0), stop=(k == NTILES - 1))
        for ct in range(CT):
            for j in range(4):
                nc.vector.tensor_scalar(
                    out=m16[ct][j * 32:(j + 1) * 32, :],
                    in0=m_ps[ct][j * 32:(j + 1) * 32, j * 32:(j + 1) * 32],
                    scalar1=1.0, scalar2=None, op0=MULT)

    at16 = []
    with tc.tile_pool(name="atps", bufs=1, space="PSUM") as atps:
        at_ps = [atps.tile([128, C], F32, tag=f"at_ps{i}", name=f"at_ps{i}")
                 for i in range(CT)]
        for h in range(NH):
            ct, j = h // 4, h % 4
            nc.tensor.matmul(at_ps[ct][j * 32:(j + 1) * 32, :],
                             m16[ct][j * 32:(j + 1) * 32, :],
                             wot16[ct][j * 32:(j + 1) * 32, :],
                             start=True, stop=True,
                             tile_position=(j * 32, j * 32))
        for ct in range(CT):
            t = pool.tile([128, C], F16, tag=f"at16_{ct}")
            nc.scalar.activation(t[:], at_ps[ct][:], AF.Copy)
            at16.append(t)

    # y: ldweights-reuse ordering (ot, ct outer; 4 col chunks inner)
    CW = HALF_PIX // 4  # 392
    with tc.tile_pool(name="yps", bufs=1, space="PSUM") as yps, \
         tc.tile_pool(name="ysb", bufs=2) as ysb:
        for ot in range(CT):
            y_ps = [yps.tile([128, CW], F32, tag=f"y_ps_{i}", name=f"y_ps{ot}_{i}")
                    for i in range(4)]
            for ct in range(CT):
                for ch in range(4):
                    nc.tensor.matmul(y_ps[ch][:], at16[ct][:, ot * 128:(ot + 1) * 128],
                                     q16[ct][:, ch * CW:(ch + 1) * CW],
                                     start=(ct == 0), stop=(ct == CT - 1))
            y_sb = ysb.tile([128, HALF_PIX], F16, tag="y_sb")
            for ch in range(4):
                nc.vector.tensor_scalar(out=y_sb[:, ch * CW:(ch + 1) * CW],
                                        in0=y_ps[ch][:], scalar1=1.0,
                                        scalar2=None, op0=MULT)
            q1.dma_start(io["y"][ot * 128:(ot + 1) * 128, :], y_sb[:])


def build_program():
    if "nc" in _CACHE:
        return _CACHE["nc"]
    nc = bacc.Bacc("TRN2", target_bir_lowering=False, debug=False, num_devices=8)
    io = {}
    io["x16"] = nc.dram_tensor("x16", (C, HP * HP), F16, kind="ExternalInput").ap()
    io["x32"] = nc.dram_tensor("x32", (C, HP * HP), F32, kind="ExternalInput").ap()
    io["xq16"] = nc.dram_tensor("xq16", (C, 30 * HP), F16, kind="ExternalInput").ap()
    io["diag3"] = nc.dram_tensor("diag3", (C, 18 * 128), F16, kind="ExternalInput").ap()
    io["misc"] = nc.dram_tensor("misc", (C, 34), F32, kind="ExternalInput").ap()
    io["wot16"] = nc.dram_tensor("wot16", (C, C), F16, kind="ExternalInput").ap()
    io["w2t32"] = nc.dram_tensor("w2t32", (C, 2), F32, kind="ExternalInput").ap()
    io["refyx"] = nc.dram_tensor("refyx", (2, N), F32, kind="ExternalInput").ap()
    io["vtab2"] = nc.dram_tensor("vtab2", (VPAD, 2 * C), F16).ap()
    io["ixy_dram"] = nc.dram_tensor("ixy_dram", (2, N), F32).ap()
    io["y"] = nc.dram_tensor("y", (C, HALF_PIX), F16, kind="ExternalOutput").ap()

    with tile.TileContext(nc) as tc:
        with contextlib.ExitStack() as ctx:
            _emit(nc, tc, ctx, io)
    nc.compile()
    _CACHE["nc"] = nc
    return nc


def host_prep(inputs):
    x = np.asarray(inputs["x"], np.float32)
    xpad32 = np.pad(x, ((0, 0), (0, 0), (1, 1), (1, 1)))
    xpad = xpad32.astype(np.float16)

    def diagblock(w):  # (C, 9) -> (C, 9*128) diagonal blocks
        w = np.asarray(w, np.float32).reshape(C, 9).astype(np.float16)
        out = np.zeros((C, 9, 128), np.float16)
        for ct in range(CT):
            rows = np.arange(128)
            out[ct * 128 + rows, :, rows] = w[ct * 128 + rows, :]
        return out.reshape(C, 9 * 128)

    shared = {}
    shared["diag3"] = np.concatenate(
        [diagblock(inputs["w_v"]), diagblock(inputs["w_k"])], axis=1)
    misc = np.zeros((C, 34), np.float32)
    misc[:, 0:9] = np.asarray(inputs["w_q"], np.float32).reshape(C, 9)
    misc[:, 9] = np.asarray(inputs["b_q"], np.float32)
    misc[:, 10] = np.asarray(inputs["b_v"], np.float32)
    misc[:, 11:20] = np.asarray(inputs["w_off1"], np.float32).reshape(C, 9)
    misc[:, 20] = np.asarray(inputs["b_k"], np.float32) * SCALE
    misc[:, 21] = np.asarray(inputs["ln_g"], np.float32)
    misc[:, 22] = np.asarray(inputs["ln_b"], np.float32)
    misc[:, 25:34] = np.asarray(inputs["w_v"], np.float32).reshape(C, 9)
    shared["misc"] = misc
    shared["wot16"] = np.ascontiguousarray(
        np.asarray(inputs["w_out"], np.float32).T).astype(np.float16)
    shared["w2t32"] = np.ascontiguousarray(
        np.asarray(inputs["w_off2"], np.float32).T)
    ry = (np.arange(KH, dtype=np.float32) + 0.5) / KH * 2 - 1
    rx = (np.arange(KW, dtype=np.float32) + 0.5) / KW * 2 - 1
    refyx = np.stack([np.repeat(ry, KW), np.tile(rx, KH)])
    shared["refyx"] = np.ascontiguousarray(refyx, dtype=np.float32)

    in_maps = []
    for core in range(8):
        b, half = core // 2, core % 2
        m = dict(shared)
        m["x16"] = np.ascontiguousarray(xpad[b].reshape(C, HP * HP))
        m["x32"] = np.ascontiguousarray(xpad32[b].reshape(C, HP * HP))
        r0 = half * HALF_ROWS
        m["xq16"] = np.ascontiguousarray(
            xpad[b, :, r0:r0 + 30, :].reshape(C, 30 * HP))
        in_maps.append(m)
    return in_maps


def assemble(results):
    y = np.empty((B, C, H, W), np.float32)
    for core in range(8):
        b, half = core // 2, core % 2
        part = results[core]["y"].astype(np.float32).reshape(C, HALF_ROWS, W)
        y[b, :, half * HALF_ROWS:(half + 1) * HALF_ROWS, :] = part
    return y


def run(inputs, trace=False):
    nc = build_program()
    in_maps = host_prep(inputs)
    res = run_bass_kernel_spmd(nc, in_maps, core_ids=list(range(8)), trace=trace)
    return assemble(res.results), res


def kernel(**inputs):
    out, _ = run(inputs, trace=False)
    return out
